# revision 1
# baseline (speedup 1.0000x reference)
"""Sliding-window GQA causal self-attention for Trainium2, 8 NeuronCores.

Sharding: 8 cores = 4 batches x 2 head-shards. Each core handles one batch
and 2 of the 4 KV groups (8 of 16 Q heads). Core computes a full [C, T]
partial of the output projection; host sums the two shards per batch.

On-core layouts (T = 1024 tokens of one batch):
  xt   [C, T]          x^T, contraction operand for all projections
  qTf  4 x [128, T]    roped+rms'd q^T; tile r rows = [head(g0,r) 64d ; head(g1,r) 64d]
  kTf  [128, T]        roped k^T (rms folded into the exp scale)
  v    [128, 8, 128]   v natural, v[p, j, c] = v[t=128j+p, ch], ch = 64*gg + d
  probs^T per (r, gg, kblock j): [128 kpos, <=384 qpos], band-masked exp(scores^T)
  y^T  4 x [128, T]    attention out, same row layout as qTf
  outT [C, T]          partial output projection (host sums shard pair, transposes)

All matmul operands are float32r (tf32-like, ~2e-4 rel err, full PE rate at
moving-dim >= 256).
"""
import numpy as np

B, T, C = 4, 1024, 1024
H, HKV, D = 16, 4, 64
REP = H // HKV
WINDOW = 256
GATE_CH = 12
NCORES = 8
EPS = float(np.finfo(np.float32).eps)
QK_SCALE = 1.2 * 1.2 / 8.0  # the two rms scales (1.2 each) * 1/sqrt(D)

_CACHE = {}


def _build_program(debug=False, reps=1):
    from contextlib import ExitStack
    import concourse.bass as bass
    import concourse.tile as tile
    from concourse import bacc, mybir
    from concourse.masks import make_identity

    f32 = mybir.dt.float32
    f32r = mybir.dt.float32r
    ts = bass.ts

    nc = bacc.Bacc("TRN2", target_bir_lowering=False, debug=False,
                   enable_asserts=True, num_devices=NCORES)

    def din(name, shape, dt=f32):
        return nc.dram_tensor(name, shape, dt, kind="ExternalInput").ap()

    xt = din("xt", [C, T], f32r)
    wq = din("wq", [C, 512], f32r)
    wk = din("wk", [C, 128], f32r)
    wv = din("wv", [C, 128], f32r)
    wo = din("wo", [512, C], f32r)
    wg = din("wg", [16, 2], f32r)        # zero-padded from 12 gate channels
    vet = din("vet", [128, T])           # 3 * ve^T rows [g0 64d ; g1 64d]
    cosb = din("cosb", [128, T])
    sinbw = din("sinbw", [128, T])       # swap32(sinb): u = z*sinbw, t2 = swap(u)
    indq8 = din("indq8", [128, 4, 8], f32r)  # [:, r, 2r+gg] = 1/64 (block rows)
    indqn = din("indqn", [128, 2], f32r)     # block indicator / 64
    indb = din("indb", [2, 128], f32r)       # block-broadcast rows, value 1
    ind018 = din("ind018", [8, 4, 128], f32r)  # [2r+gg, r, m]=QK_SCALE, gg=m//64
    onesg = din("onesg", [128, 2, 2], f32r)  # [:, gg, gg] = 1 else 0
    epsb = din("epsb", [128, 1])             # rms epsilon
    outT = nc.dram_tensor("out_t", [C, T], f32, kind="ExternalOutput").ap()
    dbg = {}
    if debug:
        for nm, shp in [("d_qTf", [512, T]), ("d_kTf", [128, T]),
                        ("d_v", [128, 8, 128]),
                        ("d_rsq", [8, T]), ("d_yTf", [512, T])]:
            dbg[nm] = nc.dram_tensor(nm, shp, f32, kind="ExternalOutput").ap()

    Exp = mybir.ActivationFunctionType.Exp
    Sqrt = mybir.ActivationFunctionType.Sqrt
    Sigmoid = mybir.ActivationFunctionType.Sigmoid
    Square = mybir.ActivationFunctionType.Square
    Copy = mybir.ActivationFunctionType.Copy
    is_ge = mybir.AluOpType.is_ge

    def rope_swap(dst, src):
        # dst[p] = src[p +/- 32] within each 64-row head block
        nc.sync.dma_start(dst[0:32, :], src[32:64, :])
        nc.sync.dma_start(dst[32:64, :], src[0:32, :])
        nc.sync.dma_start(dst[64:96, :], src[96:128, :])
        nc.sync.dma_start(dst[96:128, :], src[64:96, :])

    with tile.TileContext(nc) as tc:
     for _rep in range(reps):
      with ExitStack() as ctx:
        sing = ctx.enter_context(tc.tile_pool(name="sing", bufs=1))

        # ---------- persistent tiles ----------
        wo_sb = sing.tile([128, 4, C], f32r, name="wo_sb")
        for kr in range(4):
            nc.sync.dma_start(wo_sb[:, kr, :], wo[ts(kr, 128), :])
        indq8_sb = sing.tile([128, 4, 8], f32r, name="indq8_sb")
        nc.sync.dma_start(indq8_sb[:], indq8[:])
        indqn_sb = sing.tile([128, 2], f32r, name="indqn_sb")
        nc.sync.dma_start(indqn_sb[:], indqn[:])
        indb_sb = sing.tile([2, 128], f32r, name="indb_sb")
        nc.sync.dma_start(indb_sb[:], indb[:])
        ind018_sb = sing.tile([8, 4, 128], f32r, name="ind018_sb")
        nc.sync.dma_start(ind018_sb[:], ind018[:])
        onesg_sb = sing.tile([128, 2, 2], f32r, name="onesg_sb")
        nc.sync.dma_start(onesg_sb[:], onesg[:])
        epsb_sb = sing.tile([128, 1], f32, name="epsb_sb")
        nc.sync.dma_start(epsb_sb[:], epsb[:])
        ident = sing.tile([128, 128], f32, name="ident")
        make_identity(nc, ident[:])

        qTf = [sing.tile([128, T], f32r, name=f"qTf{r}") for r in range(4)]
        kTf = sing.tile([128, T], f32r, name="kTf")
        v_sb = sing.tile([128, 8, 128], f32r, name="v_sb")
        yTf = [sing.tile([128, T], f32r, name=f"yTf{r}") for r in range(4)]
        rsq_sb = sing.tile([8, T], f32r, name="rsq_sb")

        # ================= Stage A: projections / rope / rms / gate =========
        with tc.tile_pool(name="stA", bufs=2) as stA, \
             tc.tile_pool(name="pA_", bufs=1, space="PSUM") as pA_:
            xt_sb = stA.tile([128, 8, T], f32r, name="xt_sb", bufs=1)
            for kc in range(8):
                nc.sync.dma_start(xt_sb[:, kc, :], xt[ts(kc, 128), :])
            wq_sb = stA.tile([128, 8, 512], f32r, name="wq_sb", bufs=1)
            wk_sb = stA.tile([128, 8, 128], f32r, name="wk_sb", bufs=1)
            wv_sb = stA.tile([128, 8, 128], f32r, name="wv_sb", bufs=1)
            for kc in range(8):
                nc.sync.dma_start(wq_sb[:, kc, :], wq[ts(kc, 128), :])
                nc.sync.dma_start(wk_sb[:, kc, :], wk[ts(kc, 128), :])
                nc.sync.dma_start(wv_sb[:, kc, :], wv[ts(kc, 128), :])
            wg_sb = stA.tile([16, 2], f32r, name="wg_sb", bufs=1)
            nc.sync.dma_start(wg_sb[:], wg[:])
            vet_sb = stA.tile([128, T], f32, name="vet_sb", bufs=1)
            nc.sync.dma_start(vet_sb[:], vet[:])
            cosb_sb = stA.tile([128, T], f32, name="cosb_sb", bufs=1)
            nc.sync.dma_start(cosb_sb[:], cosb[:])
            sinbw_sb = stA.tile([128, T], f32, name="sinbw_sb", bufs=1)
            nc.sync.dma_start(sinbw_sb[:], sinbw[:])

            for h in range(2):
                tsl = slice(512 * h, 512 * h + 512)

                # ---- projections, streamed over xt chunks: k, v, q0, q1 first
                k_ps = pA_.tile([128, 512], f32, name="k_ps", tag="kps")
                v_ps = pA_.tile([128, 512], f32, name="v_ps", tag="vps")
                q_ps01 = [pA_.tile([128, 512], f32, name=f"q_ps{r}", tag="qps",
                                   bufs=2) for r in range(2)]
                for kc in range(8):
                    st, sp = kc == 0, kc == 7
                    nc.tensor.matmul(k_ps[:], wk_sb[:, kc, :], xt_sb[:, kc, tsl],
                                     start=st, stop=sp)
                    nc.tensor.matmul(v_ps[:], wv_sb[:, kc, :], xt_sb[:, kc, tsl],
                                     start=st, stop=sp)
                    for r in range(2):
                        nc.tensor.matmul(q_ps01[r][:], wq_sb[:, kc, ts(r, 128)],
                                         xt_sb[:, kc, tsl], start=st, stop=sp)
                g_ps = pA_.tile([2, 512], f32, name="g_ps", tag="mix")
                nc.tensor.matmul(g_ps[:], wg_sb[:], xt_sb[0:16, 0, tsl],
                                 start=True, stop=True)

                # ---- gate + value-embedding; v' = v + (3*sigmoid(g)) * ve
                sig_sb = stA.tile([2, 512], f32r, name="sig_sb", tag="sig")
                nc.scalar.activation(sig_sb[:], g_ps[:], Sigmoid)
                gb_ps = pA_.tile([128, 512], f32, name="gb_ps", tag="mix")
                nc.tensor.matmul(gb_ps[:], indb_sb[:], sig_sb[:],
                                 start=True, stop=True)
                gve_sb = stA.tile([128, 512], f32, name="gve_sb", tag="gve")
                nc.vector.tensor_mul(gve_sb[:], gb_ps[:], vet_sb[:, tsl])
                vp_sb = stA.tile([128, 512], f32, name="vp_sb", tag="vp")
                nc.vector.tensor_add(vp_sb[:], v_ps[:], gve_sb[:])
                for tb in range(4):
                    vt_ps = pA_.tile([128, 128], f32, name="vt_ps", tag="mix")
                    nc.tensor.transpose(vt_ps[:], vp_sb[:, ts(tb, 128)], ident[:])
                    nc.vector.tensor_copy(v_sb[:, 4 * h + tb, :], vt_ps[:])

                # ---- k: rope into kTf, then fold rstd_k into kTf
                ku_sb = stA.tile([128, 512], f32, name="ku_sb", tag="sw")
                nc.vector.tensor_mul(ku_sb[:], k_ps[:], sinbw_sb[:, tsl])
                ksw_sb = stA.tile([128, 512], f32, name="ksw_sb", tag="sw2")
                rope_swap(ksw_sb, ku_sb)
                nc.vector.tensor_mul(kTf[:, tsl], k_ps[:], cosb_sb[:, tsl])
                nc.vector.tensor_add(kTf[:, tsl], kTf[:, tsl], ksw_sb[:])
                k2_sb = stA.tile([128, 512], f32r, name="k2_sb", tag="sq2")
                nc.vector.tensor_mul(k2_sb[:], kTf[:, tsl], kTf[:, tsl])
                mskr_ps = pA_.tile([2, 512], f32, name="mskr_ps", tag="mskr")
                nc.tensor.matmul(mskr_ps[:], indqn_sb[:], k2_sb[:],
                                 start=True, stop=True)
                sk1 = stA.tile([2, 512], f32, name="sk1", tag="sk1")
                nc.scalar.activation(sk1[:], mskr_ps[:], Sqrt,
                                     bias=epsb_sb[0:2, :])
                rk_sb = stA.tile([2, 512], f32r, name="rk_sb", tag="rk")
                with nc.allow_low_precision("f32r rstd_k"):
                    nc.vector.reciprocal(rk_sb[:], sk1[:])
                rkb_ps = pA_.tile([128, 512], f32, name="rkb_ps", tag="mix")
                nc.tensor.matmul(rkb_ps[:], indb_sb[:], rk_sb[:],
                                 start=True, stop=True)
                nc.vector.tensor_mul(kTf[:, tsl], kTf[:, tsl], rkb_ps[:])

                # ---- q: rope into qTf + mean-square, r = 0,1 then 2,3
                msq_ps = pA_.tile([8, 512], f32, name="msq_ps", tag="msq",
                                  bufs=2)

                def do_q_rope(r, q_ps_r):
                    qu_sb = stA.tile([128, 512], f32, name="qu_sb", tag="sw")
                    nc.vector.tensor_mul(qu_sb[:], q_ps_r[:], sinbw_sb[:, tsl])
                    qsw_sb = stA.tile([128, 512], f32, name="qsw_sb", tag="sw2")
                    rope_swap(qsw_sb, qu_sb)
                    nc.vector.tensor_mul(qTf[r][:, tsl], q_ps_r[:],
                                         cosb_sb[:, tsl])
                    nc.vector.tensor_add(qTf[r][:, tsl], qTf[r][:, tsl],
                                         qsw_sb[:])
                    q2_sb = stA.tile([128, 512], f32r, name="q2_sb", tag="sq2")
                    nc.scalar.activation(q2_sb[:], qTf[r][:, tsl], Square)
                    nc.tensor.matmul(msq_ps[0:8, :], indq8_sb[:, r, :], q2_sb[:],
                                     start=(r == 0), stop=(r == 3),
                                     skip_group_check=True)

                for r in range(2):
                    do_q_rope(r, q_ps01[r])
                q_ps23 = [pA_.tile([128, 512], f32, name=f"q_ps{r}", tag="qps",
                                   bufs=2) for r in (2, 3)]
                for kc in range(8):
                    for i, r in enumerate((2, 3)):
                        nc.tensor.matmul(q_ps23[i][:], wq_sb[:, kc, ts(r, 128)],
                                         xt_sb[:, kc, tsl], start=(kc == 0),
                                         stop=(kc == 7))
                for i, r in enumerate((2, 3)):
                    do_q_rope(r, q_ps23[i])

                # ---- rstd(q) for this half, then apply rms to q in place
                sq1 = stA.tile([8, 512], f32, name="sq1", tag="sq1")
                nc.scalar.activation(sq1[:], msq_ps[:], Sqrt, bias=epsb_sb[0:8, :])
                with nc.allow_low_precision("f32r rstd"):
                    nc.vector.reciprocal(rsq_sb[:, tsl], sq1[:])
                for r in range(4):
                    rb_ps = pA_.tile([128, 512], f32, name="rb_ps", tag="mix")
                    nc.tensor.matmul(rb_ps[:], ind018_sb[:, r, :], rsq_sb[:, tsl],
                                     start=True, stop=True)
                    nc.vector.tensor_mul(qTf[r][:, tsl], qTf[r][:, tsl], rb_ps[:])


        # ================= Stage B: attention ================================
        with tc.tile_pool(name="stB", bufs=2) as stB, \
             tc.tile_pool(name="pB_", bufs=1, space="PSUM") as pB_:
            for r in range(4):
                pkeep = {j: stB.tile([128, 2, 384], f32r, name=f"pk{j}",
                                     tag=f"pk{j}", bufs=2) for j in (2, 3)}
                for h in range(2):
                    tsl = slice(512 * h, 512 * h + 512)
                    y_ps = [pB_.tile([64, 512], f32, name=f"y_ps{gg}",
                                     tag=f"yps{gg}", bufs=1) for gg in range(2)]
                    sums_ps = pB_.tile([2, 512], f32, name="sums_ps", tag="sums")
                    jlist = list(range(0, 4)) if h == 0 else list(range(2, 8))
                    first = True
                    for j in jlist:
                        w = min(384, T - 128 * j)
                        if h == 1 and j in pkeep:
                            p2 = pkeep[j]          # cached from h == 0
                        else:
                            if j in pkeep:
                                p2 = pkeep[j]
                            else:
                                p2 = stB.tile([128, 2, 384], f32r, name="p2",
                                              tag="p2", bufs=3)
                            sc2 = pB_.tile([128, 2, 512], f32, name="sc2",
                                           tag="sc", bufs=2)
                            nc.tensor.matmul(sc2[:, 0, 0:w],
                                             kTf[0:64, ts(j, 128)],
                                             qTf[r][0:64, 128 * j:128 * j + w],
                                             start=True, stop=True)
                            nc.tensor.matmul(sc2[:, 1, 0:w],
                                             kTf[64:128, ts(j, 128)],
                                             qTf[r][64:128, 128 * j:128 * j + w],
                                             start=True, stop=True)
                            nc.scalar.activation(p2[:, :, 0:w], sc2[:, :, 0:w],
                                                 Exp)
                            wl = min(256, w)
                            # keep cols [0, wl) where i - p >= 0 (causal edge)
                            nc.gpsimd.affine_select(
                                p2[:, :, 0:wl], p2[:, :, 0:wl], compare_op=is_ge,
                                fill=0.0, base=0, pattern=[[0, 2], [1, wl]],
                                channel_multiplier=-1)
                            if w > 256:
                                # keep cols [256, w) where p - i' >= 0 (window)
                                nc.gpsimd.affine_select(
                                    p2[:, :, 256:w], p2[:, :, 256:w],
                                    compare_op=is_ge, fill=0.0, base=0,
                                    pattern=[[0, 2], [-1, w - 256]],
                                    channel_multiplier=1)
                        a = max(128 * j, 512 * h)
                        b = min(128 * j + w, 512 * h + 512)
                        n0, nn = a - 128 * j, b - a
                        for gg in range(2):
                            nc.tensor.matmul(
                                y_ps[gg][:, a - 512 * h:b - 512 * h],
                                v_sb[:, j, ts(gg, 64)], p2[:, gg, n0:n0 + nn],
                                start=first, stop=(j == jlist[-1]),
                                skip_group_check=True)
                            nc.tensor.matmul(
                                sums_ps[:, a - 512 * h:b - 512 * h],
                                onesg_sb[:, gg, :], p2[:, gg, n0:n0 + nn],
                                start=(first and gg == 0),
                                stop=(j == jlist[-1] and gg == 1),
                                skip_group_check=True)
                        first = False
                    # normalize y by 1/sums (broadcast over 64 d rows per head)
                    rsum_sb = stB.tile([2, 512], f32r, name="rsum_sb",
                                       tag="rsum")
                    with nc.allow_low_precision("f32r 1/sums"):
                        nc.vector.reciprocal(rsum_sb[:], sums_ps[:])
                    rbs_ps = pB_.tile([128, 512], f32, name="rbs_ps", tag="rbs")
                    nc.tensor.matmul(rbs_ps[:], indb_sb[:], rsum_sb[:],
                                     start=True, stop=True)
                    rbs_sb = stB.tile([128, 512], f32, name="rbs_sb",
                                      tag="rbs_sb")
                    if h == 0:
                        nc.scalar.activation(rbs_sb[:], rbs_ps[:], Copy)
                    else:
                        nc.vector.tensor_copy(rbs_sb[:], rbs_ps[:])
                    for gg in range(2):
                        nc.vector.tensor_mul(yTf[r][ts(gg, 64), tsl],
                                             y_ps[gg][:], rbs_sb[ts(gg, 64), :])


        # ================= Stage C: output projection ========================
        with tc.tile_pool(name="stC", bufs=3) as stC, \
             tc.tile_pool(name="pC_", bufs=2, space="PSUM") as pC_:
            for ct in range(8):
                for h in range(2):
                    tsl = slice(512 * h, 512 * h + 512)
                    o_ps = pC_.tile([128, 512], f32, name="o_ps", tag="ops")
                    for kr in range(4):
                        nc.tensor.matmul(o_ps[:], wo_sb[:, kr, ts(ct, 128)],
                                         yTf[kr][:, tsl], start=(kr == 0),
                                         stop=(kr == 3))
                    o_sb = stC.tile([128, 512], f32, name="o_sb", tag="osb")
                    if (ct + h) % 2 == 0:
                        nc.vector.tensor_copy(o_sb[:], o_ps[:])
                    else:
                        nc.scalar.activation(o_sb[:], o_ps[:], Copy)
                    nc.sync.dma_start(outT[ts(ct, 128), tsl], o_sb[:])

    nc.compile()
    return nc


def _const_inputs():
    indq8 = np.zeros((128, 4, 8), dtype=np.float32)
    for r in range(4):
        indq8[0:64, r, 2 * r] = 1.0 / D
        indq8[64:128, r, 2 * r + 1] = 1.0 / D
    indqn = np.zeros((128, 2), dtype=np.float32)
    indqn[0:64, 0] = 1.0 / D
    indqn[64:128, 1] = 1.0 / D
    indb = np.zeros((2, 128), dtype=np.float32)
    indb[0, 0:64] = 1.0
    indb[1, 64:128] = 1.0
    ind018 = np.zeros((8, 4, 128), dtype=np.float32)
    for r in range(4):
        ind018[2 * r, r, 0:64] = QK_SCALE
        ind018[2 * r + 1, r, 64:128] = QK_SCALE
    onesg = np.zeros((128, 2, 2), dtype=np.float32)
    onesg[:, 0, 0] = 1.0
    onesg[:, 1, 1] = 1.0
    epsb = np.full((128, 1), EPS, dtype=np.float32)
    return dict(indq8=indq8, indqn=indqn, indb=indb, ind018=ind018,
                onesg=onesg, epsb=epsb)


def _prep_core_inputs(x, ve3, cosb, sinb, sinbw, Wq, Wk, Wv, Wo, Wg, consts, b, s):
    g0, g1 = 2 * s, 2 * s + 1
    xt = np.ascontiguousarray(x[b].T)

    Wq4 = Wq.reshape(HKV, REP, D, C)
    wq_rows = np.concatenate([Wq4[g, r] for r in range(REP) for g in (g0, g1)],
                             axis=0)                       # (512, C)
    wq = np.ascontiguousarray(wq_rows.T)                   # (C, 512)
    Wk3 = Wk.reshape(HKV, D, C)
    wk = np.ascontiguousarray(np.concatenate([Wk3[g0], Wk3[g1]], axis=0).T)
    Wv3 = Wv.reshape(HKV, D, C)
    wv = np.ascontiguousarray(np.concatenate([Wv3[g0], Wv3[g1]], axis=0).T)

    Wo4 = Wo.reshape(C, HKV, REP, D)
    wo_cols = np.concatenate([Wo4[:, g, r, :] for r in range(REP)
                              for g in (g0, g1)], axis=1)  # (C, 512)
    wo = np.ascontiguousarray(wo_cols.T)                   # (512, C)

    wg = np.zeros((16, 2), dtype=np.float32)
    wg[0:GATE_CH, 0] = Wg[g0]
    wg[0:GATE_CH, 1] = Wg[g1]

    ve4 = ve3[b].reshape(T, HKV, D)
    vet = np.ascontiguousarray(
        np.concatenate([ve4[:, g0, :], ve4[:, g1, :]], axis=1).T)  # (128, T)

    d = dict(xt=xt, wq=wq, wk=wk, wv=wv, wo=wo, wg=wg, vet=vet,
             cosb=cosb, sinbw=sinbw)
    d.update(consts)
    return d


def kernel(x, ve, cos, sin, Wq, Wk, Wv, Wo, Wg, window_size):
    from concourse.bass_utils import run_bass_kernel_spmd

    assert int(window_size) == WINDOW
    x = np.asarray(x, dtype=np.float32)
    ve = np.asarray(ve, dtype=np.float32)
    Wq = np.asarray(Wq, dtype=np.float32)
    Wk = np.asarray(Wk, dtype=np.float32)
    Wv = np.asarray(Wv, dtype=np.float32)
    Wo = np.asarray(Wo, dtype=np.float32)
    Wg = np.asarray(Wg, dtype=np.float32)
    c = np.asarray(cos, dtype=np.float32).reshape(T, D // 2)   # (T, 32)
    sn = np.asarray(sin, dtype=np.float32).reshape(T, D // 2)

    cosb = np.ascontiguousarray(np.tile(c.T, (4, 1)))          # (128, T)
    sinb = np.ascontiguousarray(
        np.concatenate([sn.T, -sn.T, sn.T, -sn.T], axis=0))    # (128, T)
    sinbw = np.ascontiguousarray(
        np.concatenate([-sn.T, sn.T, -sn.T, sn.T], axis=0))    # swap32 rows
    ve3 = 3.0 * ve
    consts = _const_inputs()

    if "nc" not in _CACHE:
        _CACHE["nc"] = _build_program()
    nc = _CACHE["nc"]

    in_maps = []
    for core in range(NCORES):
        b, s = core // 2, core % 2
        in_maps.append(_prep_core_inputs(x, ve3, cosb, sinb, sinbw,
                                         Wq, Wk, Wv, Wo, Wg, consts, b, s))

    res = run_bass_kernel_spmd(nc, in_maps, core_ids=list(range(NCORES)))
    out = np.empty((B, T, C), dtype=np.float32)
    for b in range(B):
        acc = res.results[2 * b]["out_t"] + res.results[2 * b + 1]["out_t"]
        out[b] = acc.T
    return out



# revision 22
# speedup vs baseline: 1.2065x; 1.2065x over previous
"""Sliding-window GQA causal self-attention for Trainium2, 8 NeuronCores.

Sharding: 8 cores = 4 batches x 2 head-shards. Each core handles one batch
and 2 of the 4 KV groups (8 of 16 Q heads). Core computes a full [C, T]
partial of the output projection; host sums the two shards per batch.

v2 design (vs v1 baseline):
  - rope pairs (d, d+32) interleaved as adjacent SBUF rows host-side, so the
    rope "swap" is one DVE stream_shuffle (mask [1,0,3,2,...]) instead of 4
    SBUF-SBUF DMAs per tensor chunk.
  - rms stats computed from PRE-rope q/k (rotation preserves norms), off the
    critical path.
  - rstd = exp(-0.5*ln(ms+eps)) on Act: single activation table
    (natural_log_exp_and_others) for the whole program; gate sigmoid via Exp.
  - k's rstd folded into the softmax Exp as a per-partition (per-kpos) scale.
  - softmax denominators via an extra all-ones column in the PV stationary
    (row 64 of y PSUM = sum of probs): no PE sums matmuls.
  - sliding-window mask as a single bf16 mask-tile multiply per score block
    (DVE 2-byte fast path) instead of gpsimd affine_selects.
  - q/k/p2/v in bf16 (scores/softmax matmuls at 1 cyc/row at any width),
    projections and out-proj in f32r.
  - denominators broadcast with gpsimd partition_broadcast (no PE).
  - single-DMA-per-tensor input loads; output staged in SBUF, few bf16 DMAs.
  - phases software-pipelined in issue order: A(h0), B(h0), A(h1),
    B(h1)|C(h0), C(h1), sharing one 8-bank PSUM budget.

Layouts (T = 1024 tokens of one batch):
  xt_sb  [128, 8, T]    x^T, chunk-major; contraction operand for projections
  qTf    4 x [128, T]   bf16 roped+rms'd q^T; tile r rows = [head(g0,r); head(g1,r)]
  kTf    [128, T]       bf16 roped k^T (rstd_k folded into Exp scale)
  v_sb   [128, 8, 130]  bf16 v' = v + gate*ve, kpos-partition; per gg 64 cols
                        + all-ones col (softmax denominator)
  p2     per (r, j): [128, 2, 384] bf16 masked exp(scores^T)
  yTf    4 x [128, T]   f32r normalized attention out, rows as qTf
  ostage [128, 8, T]    bf16 output projection partial, host sums shard pair
"""
import numpy as np

B, T, C = 4, 1024, 1024
H, HKV, D = 16, 4, 64
REP = H // HKV
WINDOW = 256
GATE_CH = 12
NCORES = 8
EPS = float(np.finfo(np.float32).eps)
QK_SCALE = 1.2 * 1.2 / 8.0  # two rms scales (1.2 each) * 1/sqrt(D)

# per-h interleave of score blocks (qk) and PV accumulation, one block lag
_B_ORDER = {
    0: [("qk", 0), ("qk", 1), ("pv", 0), ("qk", 2), ("pv", 1), ("qk", 3),
        ("pv", 2), ("pv", 3)],
    1: [("qk", 2), ("qk", 3), ("pv", 2), ("qk", 4), ("pv", 3), ("qk", 5),
        ("pv", 4), ("qk", 6), ("pv", 5), ("qk", 7), ("pv", 6), ("pv", 7)],
}

_CACHE = {}


def _build_program(debug=False, reps=1):
    from contextlib import ExitStack
    import concourse.bass as bass
    import concourse.tile as tile
    from concourse import bacc, mybir
    from concourse.masks import make_identity

    f32 = mybir.dt.float32
    f32r = mybir.dt.float32r
    bf16 = mybir.dt.bfloat16
    ts = bass.ts

    nc = bacc.Bacc("TRN2", target_bir_lowering=False, debug=False,
                   enable_asserts=True, num_devices=NCORES)

    def din(name, shape, dt=f32r):
        return nc.dram_tensor(name, shape, dt, kind="ExternalInput").ap()

    xt = din("xt", [128, 8, T])            # x^T chunk-major f32r
    wq = din("wq", [128, 8, 512])          # f32r, rope-interleaved rows
    wk = din("wk", [128, 8, 128])          # f32r, rope-interleaved rows
    wv = din("wv", [128, 8, 128])          # f32r natural rows
    wo = din("wo", [128, 4, C])            # f32r
    wg = din("wg", [16, 2])                # f32r, zero-padded gate weights
    vet = din("vet", [128, 8, 128], bf16)  # 3*ve, t-partition per j block
    cosb = din("cosb", [128, T], bf16)
    sinp = din("sinp", [128, T], bf16)     # pre-shuffled signed sin table
    maskc = din("maskc", [128, 384], bf16)  # band mask per score block
    indq8 = din("indq8", [128, 4, 8], bf16)
    ind2g = din("ind2g", [128, 2], bf16)
    indb = din("indb", [33, 128])          # f32r block-broadcast rows
    ind018 = din("ind018", [8, 4, 128])    # f32r, QK_SCALE block rows
    epsb = din("epsb", [128, 1], f32)
    outb = nc.dram_tensor("outb", [128, 8, T], bf16, kind="ExternalOutput").ap()
    dbg = {}
    if debug:
        for nm, shp, dt in [("d_qTf", [512, T], bf16),
                            ("d_kTf", [128, T], bf16),
                            ("d_vsb", [128, 8, 130], bf16),
                            ("d_rsq", [8, T], f32r),
                            ("d_rkt", [128, 4, 4], f32),
                            ("d_yTf", [512, T], f32r)]:
            dbg[nm] = nc.dram_tensor(nm, shp, dt, kind="ExternalOutput").ap()

    Exp = mybir.ActivationFunctionType.Exp
    Ln = mybir.ActivationFunctionType.Ln
    Copy = mybir.ActivationFunctionType.Copy
    mult = mybir.AluOpType.mult
    add = mybir.AluOpType.add
    SHUF = [i ^ 1 for i in range(32)]      # swap rope pairs within quadrants

    with tile.TileContext(nc) as tc:
     for _rep in range(reps):
      with ExitStack() as ctx:
        sing = ctx.enter_context(tc.tile_pool(name="sing", bufs=1))
        work = ctx.enter_context(tc.tile_pool(name="work", bufs=2))
        pmm = ctx.enter_context(tc.tile_pool(name="pmm", bufs=2, space="PSUM"))
        psc = ctx.enter_context(tc.tile_pool(name="psc", bufs=3, space="PSUM"))
        pyy = ctx.enter_context(tc.tile_pool(name="pyy", bufs=2, space="PSUM"))
        pms = ctx.enter_context(tc.tile_pool(name="pms", bufs=1, space="PSUM"))

        # ---------------- persistent SBUF tiles + input DMAs ----------------
        # issue order tuned so first k-projection matmuls can start early
        xt_sb = sing.tile([128, 8, T], f32r, name="xt_sb")
        wk_sb = sing.tile([128, 8, 128], f32r, name="wk_sb")
        wv_sb = sing.tile([128, 8, 128], f32r, name="wv_sb")
        wq_sb = sing.tile([128, 8, 512], f32r, name="wq_sb")
        nc.sync.dma_start(xt_sb[:, 0:2, :], xt[:, 0:2, :])
        nc.sync.dma_start(wk_sb[:], wk[:])
        nc.sync.dma_start(xt_sb[:, 2:4, :], xt[:, 2:4, :])
        nc.sync.dma_start(wv_sb[:], wv[:])
        nc.sync.dma_start(xt_sb[:, 4:6, :], xt[:, 4:6, :])
        nc.sync.dma_start(wq_sb[:], wq[:])
        nc.sync.dma_start(xt_sb[:, 6:8, :], xt[:, 6:8, :])
        cosb_sb = sing.tile([128, T], bf16, name="cosb_sb")
        nc.sync.dma_start(cosb_sb[:], cosb[:])
        sinp_sb = sing.tile([128, T], bf16, name="sinp_sb")
        nc.sync.dma_start(sinp_sb[:], sinp[:])
        maskc_sb = sing.tile([128, 384], bf16, name="maskc_sb")
        nc.sync.dma_start(maskc_sb[:], maskc[:])
        indq8_sb = sing.tile([128, 4, 8], bf16, name="indq8_sb")
        nc.sync.dma_start(indq8_sb[:], indq8[:])
        ind2g_sb = sing.tile([128, 2], bf16, name="ind2g_sb")
        nc.sync.dma_start(ind2g_sb[:], ind2g[:])
        indb_sb = sing.tile([33, 128], f32r, name="indb_sb")
        nc.sync.dma_start(indb_sb[:], indb[:])
        ind018_sb = sing.tile([8, 4, 128], f32r, name="ind018_sb")
        nc.sync.dma_start(ind018_sb[:], ind018[:])
        epsb_sb = sing.tile([128, 1], f32, name="epsb_sb")
        nc.sync.dma_start(epsb_sb[:], epsb[:])
        wg_sb = sing.tile([16, 2], f32r, name="wg_sb")
        nc.sync.dma_start(wg_sb[:], wg[:])
        vet_sb = sing.tile([128, 8, 128], bf16, name="vet_sb")
        nc.sync.dma_start(vet_sb[:], vet[:])
        wo_sb = sing.tile([128, 4, C], f32r, name="wo_sb")
        nc.sync.dma_start(wo_sb[:], wo[:])

        ident = sing.tile([128, 128], f32, name="ident")
        make_identity(nc, ident[:])

        qTf = [sing.tile([128, T], bf16, name=f"qTf{r}") for r in range(4)]
        kTf = sing.tile([128, T], bf16, name="kTf")
        v_sb = sing.tile([128, 8, 130], bf16, name="v_sb")
        nc.gpsimd.memset(v_sb[:, :, 64], 1.0)
        nc.gpsimd.memset(v_sb[:, :, 129], 1.0)
        yTf = [sing.tile([128, T], f32r, name=f"yTf{r}") for r in range(4)]
        rk_t = [sing.tile([128, 2, 4], f32, name=f"rk_t{h}") for h in range(2)]
        rsq = [sing.tile([8, 512], f32r, name=f"rsq{h}") for h in range(2)]
        ostage = sing.tile([128, 8, T], bf16, name="ostage")

        # =========== phase A(h): projections, rope, rms, gate, v ============
        def phase_A(h):
            tsl = slice(512 * h, 512 * h + 512)

            def proj(dst_ps, w_sb, msl):
                for kc in range(8):
                    nc.tensor.matmul(dst_ps[:], w_sb[:, kc, msl],
                                     xt_sb[:, kc, tsl],
                                     start=(kc == 0), stop=(kc == 7))

            def rope(dst, src_bf):
                # dst = src*cos + shuffle(src*sinp); all bf16 SBUF
                u = work.tile([128, 512], bf16, name="u", tag="u")
                nc.vector.tensor_mul(u[:], src_bf[:], sinp_sb[:, tsl])
                usw = work.tile([128, 512], bf16, name="usw", tag="usw")
                nc.vector.stream_shuffle(usw[:], u[:], SHUF)
                nc.vector.tensor_mul(dst, src_bf[:], cosb_sb[:, tsl])
                nc.vector.tensor_add(dst, dst, usw[:])

            # ---- k ----
            k_ps = pmm.tile([128, 512], f32, name="k_ps", tag="mm")
            proj(k_ps, wk_sb, slice(0, 128))
            kc_bf = work.tile([128, 512], bf16, name="kc_bf", tag="kc")
            nc.scalar.activation(kc_bf[:], k_ps[:], Copy)
            k2 = work.tile([128, 512], bf16, name="k2", tag="sq")
            nc.vector.tensor_mul(k2[:], kc_bf[:], kc_bf[:])
            # ms_k transposed: out[kpos, gg] per j block, offsets {jj, 256+jj}
            msk_ps = pmm.tile([128, 2, 256], f32, name="msk_ps", tag="mm")
            for jj in range(4):
                nc.tensor.matmul(msk_ps[:, :, jj], k2[:, ts(jj, 128)],
                                 ind2g_sb[:], start=True, stop=True)
            lnk = work.tile([128, 2, 4], f32, name="lnk", tag="lnk")
            nc.scalar.activation(lnk[:], msk_ps[:, :, 0:4], Ln,
                                 bias=epsb_sb[:])
            nc.scalar.activation(rk_t[h][:], lnk[:], Exp, scale=-0.5)
            rope(kTf[:, tsl], kc_bf)

            # ---- v + gate ----
            v_ps = pmm.tile([128, 512], f32, name="v_ps", tag="mm")
            proj(v_ps, wv_sb, slice(0, 128))
            vc_sb = work.tile([128, 512], f32, name="vc_sb", tag="vc")
            nc.scalar.activation(vc_sb[:], v_ps[:], Copy)
            for jj in range(4):
                j = 4 * h + jj
                g_ps = pmm.tile([128, 512], f32, name="g_ps", tag="mm")
                # gate logits: [t-part, gg] via tiny matmul on x[0:16]
                nc.tensor.matmul(g_ps[:, 0:2], xt_sb[0:16, 0, ts(j, 128)],
                                 wg_sb[:], start=True, stop=True)
                vt_ps = pmm.tile([128, 512], f32, name="vt_ps", tag="mm")
                nc.tensor.transpose(vt_ps[:, 0:128], vc_sb[:, ts(jj, 128)],
                                    ident[:])
                eg = work.tile([128, 2], f32, name="eg", tag="eg")
                nc.scalar.activation(eg[:], g_ps[:, 0:2], Exp, scale=-1.0)
                nc.vector.tensor_scalar_add(eg[:], eg[:], 1.0)
                gs = work.tile([128, 2], f32, name="gs", tag="gs")
                nc.vector.reciprocal(gs[:], eg[:])
                for gg in range(2):
                    nc.vector.scalar_tensor_tensor(
                        v_sb[:, j, 65 * gg:65 * gg + 64],
                        vet_sb[:, j, ts(gg, 64)], gs[:, gg:gg + 1],
                        vt_ps[:, 64 * gg:64 * gg + 64],
                        op0=mult, op1=add)

            # ---- q: project r, square into msall, rope ----
            msall = pms.tile([8, 512], f32, name="msall", tag="ms")
            for r in range(4):
                q_ps = pmm.tile([128, 512], f32, name="q_ps", tag="mm")
                proj(q_ps, wq_sb, ts(r, 128))
                qc = work.tile([128, 512], bf16, name="qc", tag="qc")
                nc.scalar.activation(qc[:], q_ps[:], Copy)
                q2 = work.tile([128, 512], bf16, name="q2", tag="sq")
                nc.vector.tensor_mul(q2[:], qc[:], qc[:])
                nc.tensor.matmul(msall[:], indq8_sb[:, r, :], q2[:],
                                 start=(r == 0), stop=(r == 3),
                                 skip_group_check=True)
                rope(qTf[r][:, tsl], qc)

            # ---- rstd(q) and apply ----
            lnq = work.tile([8, 512], f32, name="lnq", tag="lnq")
            nc.scalar.activation(lnq[:], msall[:], Ln, bias=epsb_sb[0:8, :])
            nc.scalar.activation(rsq[h][:], lnq[:], Exp, scale=-0.5)
            for r in range(4):
                rb_ps = pmm.tile([128, 512], f32, name="rb_ps", tag="mm")
                nc.tensor.matmul(rb_ps[:], ind018_sb[:, r, :], rsq[h][:],
                                 start=True, stop=True)
                nc.vector.tensor_mul(qTf[r][:, tsl], qTf[r][:, tsl], rb_ps[:])

        # =================== phase B(h, r): attention =======================
        def phase_B(h, r):
            tsl = slice(512 * h, 512 * h + 512)
            jlist = list(range(0, 4)) if h == 0 else list(range(2, 8))
            y_ps = [pyy.tile([65, 512], f32, name=f"y_ps{gg}", tag="yy")
                    for gg in range(2)]
            p2s = {}

            def block_cols(j):
                # q columns of block j owned by this h-phase (local coords)
                c0 = max(0, 512 * h - 128 * j)
                c1 = min(384, 512 * h + 512 - 128 * j, T - 128 * j)
                return c0, c1

            def qk_block(j):
                c0, c1 = block_cols(j)
                jj = j % 4
                p2 = work.tile([128, 2, 384], bf16, name="p2", tag="p2",
                               bufs=3)
                for gg in range(2):
                    sc = psc.tile([128, 384], f32, name="sc", tag="sc")
                    nc.tensor.matmul(
                        sc[:, c0:c1], kTf[ts(gg, 64), ts(j, 128)],
                        qTf[r][ts(gg, 64), 128 * j + c0:128 * j + c1],
                        start=True, stop=True)
                    nc.scalar.activation(p2[:, gg, c0:c1], sc[:, c0:c1], Exp,
                                         scale=rk_t[j // 4][:, gg, jj:jj + 1])
                    nc.vector.tensor_mul(p2[:, gg, c0:c1], p2[:, gg, c0:c1],
                                         maskc_sb[:, c0:c1])
                p2s[j] = p2

            def pv_block(j, first, last):
                c0, c1 = block_cols(j)
                a, b = 128 * j + c0, 128 * j + c1
                for gg in range(2):
                    nc.tensor.matmul(
                        y_ps[gg][:, a - 512 * h:b - 512 * h],
                        v_sb[:, j, 65 * gg:65 * gg + 65],
                        p2s[j][:, gg, c0:c1],
                        start=first, stop=last, skip_group_check=True)

            for kind, j in _B_ORDER[h]:
                if kind == "qk":
                    qk_block(j)
                else:
                    pv_block(j, first=(j == jlist[0]), last=(j == jlist[-1]))

            # ---- normalize: y /= sums (row 64); recips at partitions 0/32 ----
            rs = work.tile([33, 512], f32r, name="rs", tag="rs")
            with nc.allow_low_precision("f32r 1/sums"):
                nc.vector.reciprocal(rs[0:1, :], y_ps[0][64:65, :])
                nc.vector.reciprocal(rs[32:33, :], y_ps[1][64:65, :])
            rb_ps = pmm.tile([128, 512], f32, name="rb_ps", tag="mm")
            nc.tensor.matmul(rb_ps[:], indb_sb[:], rs[:],
                             start=True, stop=True)
            rbc = work.tile([128, 512], f32, name="rbc", tag="rbc")
            nc.scalar.activation(rbc[:], rb_ps[:], Copy)
            for gg in range(2):
                nc.vector.tensor_mul(yTf[r][ts(gg, 64), tsl],
                                     y_ps[gg][0:64, :], rbc[ts(gg, 64), :])

        # ================= phase C(ct, h): output projection ================
        def phase_C(ct, h):
            tsl = slice(512 * h, 512 * h + 512)
            o_ps = pmm.tile([128, 512], f32, name="o_ps", tag="mm")
            for kr in range(4):
                nc.tensor.matmul(o_ps[:], wo_sb[:, kr, ts(ct, 128)],
                                 yTf[kr][:, tsl], start=(kr == 0),
                                 stop=(kr == 3))
            if (ct + h) % 2 == 0:
                nc.scalar.activation(ostage[:, ct, tsl], o_ps[:], Copy)
            else:
                nc.vector.tensor_copy(ostage[:, ct, tsl], o_ps[:])

        # ============================ schedule ==============================
        phase_A(0)
        for r in range(4):
            phase_B(0, r)
        phase_A(1)
        for r in range(4):
            phase_B(1, r)
            phase_C(2 * r, 0)
            phase_C(2 * r + 1, 0)
        nc.sync.dma_start(outb[:, 0:4, 0:512], ostage[:, 0:4, 0:512])
        nc.sync.dma_start(outb[:, 4:8, 0:512], ostage[:, 4:8, 0:512])
        for ct in range(8):
            phase_C(ct, 1)
            if ct % 2 == 1:
                nc.sync.dma_start(outb[:, ct - 1:ct + 1, 512:1024],
                                  ostage[:, ct - 1:ct + 1, 512:1024])

        if debug:
            for r in range(4):
                nc.sync.dma_start(dbg["d_qTf"][ts(r, 128), :], qTf[r][:])
                nc.sync.dma_start(dbg["d_yTf"][ts(r, 128), :], yTf[r][:])
            nc.sync.dma_start(dbg["d_kTf"][:], kTf[:])
            nc.sync.dma_start(dbg["d_vsb"][:], v_sb[:])
            for h in range(2):
                nc.sync.dma_start(dbg["d_rsq"][:, ts(h, 512)], rsq[h][:])
                nc.sync.dma_start(dbg["d_rkt"][:, 2 * h:2 * h + 2, :],
                                  rk_t[h][:])

    nc.compile()
    return nc


def _rope_interleave(a):
    """Reorder rows of a (64, ...) block so rope pairs (i, i+32) are adjacent
    rows (2i, 2i+1)."""
    out = np.empty_like(a)
    out[0::2] = a[0:32]
    out[1::2] = a[32:64]
    return out


def _const_inputs():
    import ml_dtypes
    bf16 = ml_dtypes.bfloat16
    p = np.arange(128)[:, None]
    c = np.arange(384)[None, :]
    maskc = (((c < 128) & (c >= p)) | ((c >= 128) & (c < 256)) |
             ((c >= 256) & (c - 256 <= p))).astype(np.float32)
    indq8 = np.zeros((128, 4, 8), dtype=np.float32)
    for r in range(4):
        indq8[0:64, r, 2 * r] = 1.0 / D
        indq8[64:128, r, 2 * r + 1] = 1.0 / D
    ind2g = np.zeros((128, 2), dtype=np.float32)
    ind2g[0:64, 0] = 1.0 / D
    ind2g[64:128, 1] = 1.0 / D
    ind018 = np.zeros((8, 4, 128), dtype=np.float32)
    for r in range(4):
        ind018[2 * r, r, 0:64] = QK_SCALE
        ind018[2 * r + 1, r, 64:128] = QK_SCALE
    indb = np.zeros((33, 128), dtype=np.float32)
    indb[0, 0:64] = 1.0
    indb[32, 64:128] = 1.0
    epsb = np.full((128, 1), EPS, dtype=np.float32)
    return dict(maskc=maskc.astype(bf16), indq8=indq8.astype(bf16),
                ind2g=ind2g.astype(bf16), ind018=ind018, indb=indb,
                epsb=epsb)


def _prep_tables(cos, sin):
    """cosb/sinp [128, T] bf16 tables with rope-interleaved row order."""
    import ml_dtypes
    bf16 = ml_dtypes.bfloat16
    c = np.asarray(cos, dtype=np.float32).reshape(T, D // 2).T   # (32, T)
    s = np.asarray(sin, dtype=np.float32).reshape(T, D // 2).T
    cos64 = np.empty((64, T), dtype=np.float32)
    cos64[0::2] = c
    cos64[1::2] = c
    sin64 = np.empty((64, T), dtype=np.float32)
    sin64[0::2] = -s     # row 2i gets partner u[2i+1]; y1 needs +sin later
    sin64[1::2] = s
    cosb = np.tile(cos64, (2, 1))
    sinp = np.tile(sin64, (2, 1))
    return cosb.astype(bf16), sinp.astype(bf16)


def _prep_core_inputs(x, ve3, cosb, sinp, Wq, Wk, Wv, Wo, Wg, consts, b, s):
    import ml_dtypes
    bf16 = ml_dtypes.bfloat16
    g0, g1 = 2 * s, 2 * s + 1
    xt = np.ascontiguousarray(
        x[b].T.reshape(8, 128, T).transpose(1, 0, 2))          # (128, 8, T)

    Wq4 = Wq.reshape(HKV, REP, D, C)
    wq_rows = np.concatenate(
        [_rope_interleave(Wq4[g, r]) for r in range(REP) for g in (g0, g1)],
        axis=0)                                                # (512, C)
    wq = np.ascontiguousarray(
        wq_rows.T.reshape(8, 128, 512).transpose(1, 0, 2))     # (128, 8, 512)
    Wk3 = Wk.reshape(HKV, D, C)
    wk_rows = np.concatenate(
        [_rope_interleave(Wk3[g]) for g in (g0, g1)], axis=0)  # (128, C)
    wk = np.ascontiguousarray(
        wk_rows.T.reshape(8, 128, 128).transpose(1, 0, 2))
    Wv3 = Wv.reshape(HKV, D, C)
    wv_rows = np.concatenate([Wv3[g0], Wv3[g1]], axis=0)
    wv = np.ascontiguousarray(
        wv_rows.T.reshape(8, 128, 128).transpose(1, 0, 2))

    Wo4 = Wo.reshape(C, HKV, REP, D)
    wo_cols = np.concatenate([Wo4[:, g, r, :] for r in range(REP)
                              for g in (g0, g1)], axis=1)      # (C, 512)
    wo = np.ascontiguousarray(
        wo_cols.T.reshape(4, 128, C).transpose(1, 0, 2))       # (128, 4, C)

    wg = np.zeros((16, 2), dtype=np.float32)
    wg[0:GATE_CH, 0] = Wg[g0]
    wg[0:GATE_CH, 1] = Wg[g1]

    ve4 = ve3[b].reshape(T, HKV, D)
    vet = np.concatenate([ve4[:, g0, :], ve4[:, g1, :]], axis=1)  # (T, 128)
    vet = np.ascontiguousarray(
        vet.reshape(8, 128, 128).transpose(1, 0, 2))              # (128, 8, 128)

    d = dict(xt=xt, wq=wq, wk=wk, wv=wv, wo=wo, wg=wg,
             vet=vet.astype(bf16), cosb=cosb, sinp=sinp)
    d.update(consts)
    return d


def kernel(x, ve, cos, sin, Wq, Wk, Wv, Wo, Wg, window_size):
    from concourse.bass_utils import run_bass_kernel_spmd

    assert int(window_size) == WINDOW
    x = np.asarray(x, dtype=np.float32)
    ve3 = 3.0 * np.asarray(ve, dtype=np.float32)
    Wq = np.asarray(Wq, dtype=np.float32)
    Wk = np.asarray(Wk, dtype=np.float32)
    Wv = np.asarray(Wv, dtype=np.float32)
    Wo = np.asarray(Wo, dtype=np.float32)
    Wg = np.asarray(Wg, dtype=np.float32)
    cosb, sinp = _prep_tables(cos, sin)
    consts = _const_inputs()

    if "nc" not in _CACHE:
        _CACHE["nc"] = _build_program()
    nc = _CACHE["nc"]

    in_maps = []
    for core in range(NCORES):
        b, s = core // 2, core % 2
        in_maps.append(_prep_core_inputs(x, ve3, cosb, sinp,
                                         Wq, Wk, Wv, Wo, Wg, consts, b, s))

    res = run_bass_kernel_spmd(nc, in_maps, core_ids=list(range(NCORES)))
    out = np.empty((B, T, C), dtype=np.float32)
    for b in range(B):
        acc = (np.asarray(res.results[2 * b]["outb"], dtype=np.float32)
               + np.asarray(res.results[2 * b + 1]["outb"], dtype=np.float32))
        acc = acc.reshape(128, 8, T).transpose(1, 0, 2).reshape(C, T)
        out[b] = acc.T
    return out


# revision 39
# speedup vs baseline: 1.3911x; 1.1530x over previous
"""Sliding-window GQA causal self-attention for Trainium2, 8 NeuronCores.

Sharding: 8 cores = 4 batches x 2 head-shards. Each core handles one batch
and 2 of the 4 KV groups (8 of 16 Q heads). Core computes a full [C, T]
partial of the output projection; host sums the two shards per batch.

v2 design (vs v1 baseline):
  - rope pairs (d, d+32) interleaved as adjacent SBUF rows host-side, so the
    rope "swap" is one DVE stream_shuffle (mask [1,0,3,2,...]) instead of 4
    SBUF-SBUF DMAs per tensor chunk.
  - rms stats computed from PRE-rope q/k (rotation preserves norms), off the
    critical path.
  - rstd = exp(-0.5*ln(ms+eps)) on Act: single activation table
    (natural_log_exp_and_others) for the whole program; gate sigmoid via Exp.
  - k's rstd folded into the softmax Exp as a per-partition (per-kpos) scale.
  - softmax denominators via an extra all-ones column in the PV stationary
    (row 64 of y PSUM = sum of probs): no PE sums matmuls.
  - sliding-window mask as a single bf16 mask-tile multiply per score block
    (DVE 2-byte fast path) instead of gpsimd affine_selects.
  - q/k/p2/v in bf16 (scores/softmax matmuls at 1 cyc/row at any width),
    projections and out-proj in f32r.
  - denominators broadcast with gpsimd partition_broadcast (no PE).
  - single-DMA-per-tensor input loads; output staged in SBUF, few bf16 DMAs.
  - phases software-pipelined in issue order: A(h0), B(h0), A(h1),
    B(h1)|C(h0), C(h1), sharing one 8-bank PSUM budget.

Layouts (T = 1024 tokens of one batch):
  xt_sb  [128, 8, T]    x^T, chunk-major; contraction operand for projections
  qTf    4 x [128, T]   bf16 roped+rms'd q^T; tile r rows = [head(g0,r); head(g1,r)]
  kTf    [128, T]       bf16 roped k^T (rstd_k folded into Exp scale)
  v_sb   [128, 8, 130]  bf16 v' = v + gate*ve, kpos-partition; per gg 64 cols
                        + all-ones col (softmax denominator)
  p2     per (r, j): [128, 2, 384] bf16 masked exp(scores^T)
  yTf    4 x [128, T]   f32r normalized attention out, rows as qTf
  ostage [128, 8, T]    bf16 output projection partial, host sums shard pair
"""
import numpy as np

B, T, C = 4, 1024, 1024
H, HKV, D = 16, 4, 64
REP = H // HKV
WINDOW = 256
GATE_CH = 12
NCORES = 8
EPS = float(np.finfo(np.float32).eps)
QK_SCALE = 1.2 * 1.2 / 8.0  # two rms scales (1.2 each) * 1/sqrt(D)

# per-h interleave of score blocks (qk) and PV accumulation, one block lag
_B_ORDER = {
    0: [("qk", 0), ("qk", 1), ("pv", 0), ("qk", 2), ("pv", 1), ("qk", 3),
        ("pv", 2), ("pv", 3)],
    1: [("qk", 2), ("qk", 3), ("pv", 2), ("qk", 4), ("pv", 3), ("qk", 5),
        ("pv", 4), ("qk", 6), ("pv", 5), ("qk", 7), ("pv", 6), ("pv", 7)],
}

_CACHE = {}


def _build_program(debug=False, reps=1):
    from contextlib import ExitStack
    import concourse.bass as bass
    import concourse.tile as tile
    from concourse import bacc, mybir
    from concourse.masks import make_identity

    f32 = mybir.dt.float32
    f32r = mybir.dt.float32r
    bf16 = mybir.dt.bfloat16
    ts = bass.ts

    nc = bacc.Bacc("TRN2", target_bir_lowering=False, debug=False,
                   enable_asserts=True, num_devices=NCORES)

    def din(name, shape, dt=f32r):
        return nc.dram_tensor(name, shape, dt, kind="ExternalInput").ap()

    xt = din("xt", [128, 8, T], bf16)      # x^T chunk-major
    wq = din("wq", [128, 8, 512], bf16)    # rope-interleaved rows
    wk = din("wk", [128, 8, 128], bf16)    # rope-interleaved rows
    wv = din("wv", [128, 8, 128], bf16)    # natural rows
    wo = din("wo", [128, 4, C], bf16)
    wg = din("wg", [16, 2], bf16)          # zero-padded gate weights
    vet = din("vet", [128, 8, 128], bf16)  # 3*ve, t-partition per j block
    cosb = din("cosb", [128, T], bf16)
    sinp = din("sinp", [128, T], bf16)     # pre-shuffled signed sin table
    indq8 = din("indq8", [128, 4, 8], bf16)
    ind2g = din("ind2g", [128, 2], bf16)
    indb = din("indb", [33, 128])          # f32r block-broadcast rows
    ind018 = din("ind018", [8, 4, 128])    # f32r, QK_SCALE block rows
    epsb = din("epsb", [128, 1], f32)
    outb = nc.dram_tensor("outb", [128, 8, T], bf16, kind="ExternalOutput").ap()
    dbg = {}
    if debug:
        for nm, shp, dt in [("d_qTf", [512, T], bf16),
                            ("d_kTf", [128, T], bf16),
                            ("d_vsb", [128, 8, 130], bf16),
                            ("d_rsq", [8, T], f32r),
                            ("d_rkt", [128, 4, 4], f32),
                            ("d_yTf", [512, T], bf16)]:
            dbg[nm] = nc.dram_tensor(nm, shp, dt, kind="ExternalOutput").ap()

    Exp = mybir.ActivationFunctionType.Exp
    Ln = mybir.ActivationFunctionType.Ln
    Copy = mybir.ActivationFunctionType.Copy
    mult = mybir.AluOpType.mult
    add = mybir.AluOpType.add
    is_ge = mybir.AluOpType.is_ge
    SHUF = [i ^ 1 for i in range(32)]      # swap rope pairs within quadrants

    with tile.TileContext(nc) as tc:
     for _rep in range(reps):
      with ExitStack() as ctx:
        sing = ctx.enter_context(tc.tile_pool(name="sing", bufs=1))
        work = ctx.enter_context(tc.tile_pool(name="work", bufs=2))
        pmm = ctx.enter_context(tc.tile_pool(name="pmm", bufs=2, space="PSUM"))
        psc = ctx.enter_context(tc.tile_pool(name="psc", bufs=3, space="PSUM"))
        pyy = ctx.enter_context(tc.tile_pool(name="pyy", bufs=2, space="PSUM"))
        pms = ctx.enter_context(tc.tile_pool(name="pms", bufs=1, space="PSUM"))

        # ---------------- persistent SBUF tiles + input DMAs ----------------
        # issue order tuned so first k-projection matmuls can start early
        xt_sb = sing.tile([128, 8, T], bf16, name="xt_sb")
        wk_sb = sing.tile([128, 8, 128], bf16, name="wk_sb")
        wv_sb = sing.tile([128, 8, 128], bf16, name="wv_sb")
        wq_sb = sing.tile([128, 8, 512], bf16, name="wq_sb")
        nc.sync.dma_start(xt_sb[:, 0:2, :], xt[:, 0:2, :])
        nc.sync.dma_start(wk_sb[:], wk[:])
        nc.sync.dma_start(xt_sb[:, 2:4, :], xt[:, 2:4, :])
        nc.sync.dma_start(wv_sb[:], wv[:])
        nc.sync.dma_start(xt_sb[:, 4:6, :], xt[:, 4:6, :])
        nc.sync.dma_start(wq_sb[:], wq[:])
        nc.sync.dma_start(xt_sb[:, 6:8, :], xt[:, 6:8, :])
        cosb_sb = sing.tile([128, T], bf16, name="cosb_sb")
        nc.sync.dma_start(cosb_sb[:], cosb[:])
        sinp_sb = sing.tile([128, T], bf16, name="sinp_sb")
        nc.sync.dma_start(sinp_sb[:], sinp[:])
        indq8_sb = sing.tile([128, 4, 8], bf16, name="indq8_sb")
        nc.sync.dma_start(indq8_sb[:], indq8[:])
        ind2g_sb = sing.tile([128, 2], bf16, name="ind2g_sb")
        nc.sync.dma_start(ind2g_sb[:], ind2g[:])
        indb_sb = sing.tile([33, 128], f32r, name="indb_sb")
        nc.sync.dma_start(indb_sb[:], indb[:])
        ind018_sb = sing.tile([8, 4, 128], f32r, name="ind018_sb")
        nc.sync.dma_start(ind018_sb[:], ind018[:])
        epsb_sb = sing.tile([128, 1], f32, name="epsb_sb")
        nc.sync.dma_start(epsb_sb[:], epsb[:])
        wg_sb = sing.tile([16, 2], bf16, name="wg_sb")
        nc.sync.dma_start(wg_sb[:], wg[:])
        vet_sb = sing.tile([128, 8, 128], bf16, name="vet_sb")
        nc.sync.dma_start(vet_sb[:], vet[:])
        wo_sb = sing.tile([128, 4, C], bf16, name="wo_sb")
        nc.sync.dma_start(wo_sb[:], wo[:])

        ident = sing.tile([128, 128], f32, name="ident")
        make_identity(nc, ident[:])

        qTf = [sing.tile([128, T], bf16, name=f"qTf{r}") for r in range(4)]
        kTf = sing.tile([128, T], bf16, name="kTf")
        v_sb = sing.tile([128, 8, 130], bf16, name="v_sb")
        nc.gpsimd.memset(v_sb[:, :, 64], 1.0)
        nc.gpsimd.memset(v_sb[:, :, 129], 1.0)
        yTf = [sing.tile([128, T], bf16, name=f"yTf{r}") for r in range(4)]
        rk_t = [sing.tile([128, 2, 4], f32, name=f"rk_t{h}") for h in range(2)]
        rsq = [sing.tile([8, 512], f32r, name=f"rsq{h}") for h in range(2)]
        ostage = sing.tile([128, 8, T], bf16, name="ostage")

        # =========== phase A(h) pieces: projections, rope, rms, gate ========
        msall_t = {}

        def _proj(h, dst_ps, w_sb, msl):
            tsl = slice(512 * h, 512 * h + 512)
            for kc in range(8):
                nc.tensor.matmul(dst_ps[:], w_sb[:, kc, msl],
                                 xt_sb[:, kc, tsl],
                                 start=(kc == 0), stop=(kc == 7))

        def _rope(h, dst, src_bf):
            # dst = src*cos + shuffle(src*sinp); all bf16 SBUF
            tsl = slice(512 * h, 512 * h + 512)
            u = work.tile([128, 512], bf16, name="u", tag="u")
            nc.vector.tensor_mul(u[:], src_bf[:], sinp_sb[:, tsl])
            usw = work.tile([128, 512], bf16, name="usw", tag="usw")
            nc.vector.stream_shuffle(usw[:], u[:], SHUF)
            nc.vector.tensor_mul(dst, src_bf[:], cosb_sb[:, tsl])
            nc.vector.tensor_add(dst, dst, usw[:])

        def A_k(h):
            tsl = slice(512 * h, 512 * h + 512)
            k_ps = pmm.tile([128, 512], f32, name="k_ps", tag="mm")
            _proj(h, k_ps, wk_sb, slice(0, 128))
            kc_bf = work.tile([128, 512], bf16, name="kc_bf", tag="kc")
            nc.scalar.activation(kc_bf[:], k_ps[:], Copy)
            k2 = work.tile([128, 512], bf16, name="k2", tag="sq")
            nc.vector.tensor_mul(k2[:], kc_bf[:], kc_bf[:])
            # ms_k transposed: out[kpos, gg] per j block, offsets {jj, 256+jj}
            msk_ps = pmm.tile([128, 2, 256], f32, name="msk_ps", tag="mm")
            for jj in range(4):
                nc.tensor.matmul(msk_ps[:, :, jj], k2[:, ts(jj, 128)],
                                 ind2g_sb[:], start=True, stop=True)
            lnk = work.tile([128, 2, 4], f32, name="lnk", tag="lnk")
            nc.scalar.activation(lnk[:], msk_ps[:, :, 0:4], Ln,
                                 bias=epsb_sb[:])
            nc.scalar.activation(rk_t[h][:], lnk[:], Exp, scale=-0.5)
            _rope(h, kTf[:, tsl], kc_bf)

        def A_v(h):
            v_ps = pmm.tile([128, 512], f32, name="v_ps", tag="mm")
            _proj(h, v_ps, wv_sb, slice(0, 128))
            vc_sb = work.tile([128, 512], f32, name="vc_sb", tag="vc")
            nc.scalar.activation(vc_sb[:], v_ps[:], Copy)
            for jj in range(4):
                j = 4 * h + jj
                g_ps = pmm.tile([128, 512], f32, name="g_ps", tag="mm")
                # gate logits: [t-part, gg] via tiny matmul on x[0:16]
                nc.tensor.matmul(g_ps[:, 0:2], xt_sb[0:16, 0, ts(j, 128)],
                                 wg_sb[:], start=True, stop=True)
                vt_ps = pmm.tile([128, 512], f32, name="vt_ps", tag="mm")
                nc.tensor.transpose(vt_ps[:, 0:128], vc_sb[:, ts(jj, 128)],
                                    ident[:])
                eg = work.tile([128, 2], f32, name="eg", tag="eg")
                nc.scalar.activation(eg[:], g_ps[:, 0:2], Exp, scale=-1.0)
                nc.gpsimd.tensor_scalar_add(eg[:], eg[:], 1.0)
                gs = work.tile([128, 2], f32, name="gs", tag="gs")
                nc.vector.reciprocal(gs[:], eg[:])
                for gg in range(2):
                    nc.vector.scalar_tensor_tensor(
                        v_sb[:, j, 65 * gg:65 * gg + 64],
                        vet_sb[:, j, ts(gg, 64)], gs[:, gg:gg + 1],
                        vt_ps[:, 64 * gg:64 * gg + 64],
                        op0=mult, op1=add)

        def A_q(h, rr):
            tsl = slice(512 * h, 512 * h + 512)
            if 0 in rr:
                msall_t[h] = pms.tile([8, 512], f32, name="msall", tag="ms")
            msall = msall_t[h]
            for r in rr:
                q_ps = pmm.tile([128, 512], f32, name="q_ps", tag="mm")
                _proj(h, q_ps, wq_sb, ts(r, 128))
                qc = work.tile([128, 512], bf16, name="qc", tag="qc")
                nc.scalar.activation(qc[:], q_ps[:], Copy)
                q2 = work.tile([128, 512], bf16, name="q2", tag="sq")
                nc.vector.tensor_mul(q2[:], qc[:], qc[:])
                nc.tensor.matmul(msall[:], indq8_sb[:, r, :], q2[:],
                                 start=(r == 0), stop=(r == 3),
                                 skip_group_check=True)
                _rope(h, qTf[r][:, tsl], qc)

        def A_tail(h):
            tsl = slice(512 * h, 512 * h + 512)
            lnq = work.tile([8, 512], f32, name="lnq", tag="lnq")
            nc.scalar.activation(lnq[:], msall_t[h][:], Ln,
                                 bias=epsb_sb[0:8, :])
            nc.scalar.activation(rsq[h][:], lnq[:], Exp, scale=-0.5)
            for r in range(4):
                rb_ps = pmm.tile([128, 512], f32, name="rb_ps", tag="mm")
                nc.tensor.matmul(rb_ps[:], ind018_sb[:, r, :], rsq[h][:],
                                 start=True, stop=True)
                nc.vector.tensor_mul(qTf[r][:, tsl], qTf[r][:, tsl], rb_ps[:])

        # =================== phase B(h, r): attention =======================
        def phase_B(h, r):
            tsl = slice(512 * h, 512 * h + 512)
            jlist = list(range(0, 4)) if h == 0 else list(range(2, 8))
            y_ps = [pyy.tile([65, 512], f32, name=f"y_ps{gg}", tag="yy")
                    for gg in range(2)]
            p2s = {}

            def block_cols(j):
                # q columns of block j owned by this h-phase (local coords)
                c0 = max(0, 512 * h - 128 * j)
                c1 = min(384, 512 * h + 512 - 128 * j, T - 128 * j)
                return c0, c1

            def qk_block(j):
                c0, c1 = block_cols(j)
                jj = j % 4
                p2 = work.tile([128, 2, 384], bf16, name="p2", tag="p2",
                               bufs=3)
                for gg in range(2):
                    sc = psc.tile([128, 384], f32, name="sc", tag="sc")
                    nc.tensor.matmul(
                        sc[:, c0:c1], kTf[ts(gg, 64), ts(j, 128)],
                        qTf[r][ts(gg, 64), 128 * j + c0:128 * j + c1],
                        start=True, stop=True)
                    nc.scalar.activation(p2[:, gg, c0:c1], sc[:, c0:c1], Exp,
                                         scale=rk_t[j // 4][:, gg, jj:jj + 1])
                # band mask on Pool (both gg at once): causal edge cols <128,
                # window edge cols >=256; middle always in-band
                ca0, ca1 = c0, min(c1, 128)
                if ca1 > ca0:
                    nc.gpsimd.affine_select(
                        p2[:, :, ca0:ca1], p2[:, :, ca0:ca1],
                        compare_op=is_ge, fill=0.0, base=ca0,
                        pattern=[[0, 2], [1, ca1 - ca0]],
                        channel_multiplier=-1)
                wa0, wa1 = max(c0, 256), c1
                if wa1 > wa0:
                    nc.gpsimd.affine_select(
                        p2[:, :, wa0:wa1], p2[:, :, wa0:wa1],
                        compare_op=is_ge, fill=0.0, base=256 - wa0,
                        pattern=[[0, 2], [-1, wa1 - wa0]],
                        channel_multiplier=1)
                p2s[j] = p2

            def pv_block(j, first, last):
                c0, c1 = block_cols(j)
                a, b = 128 * j + c0, 128 * j + c1
                for gg in range(2):
                    nc.tensor.matmul(
                        y_ps[gg][:, a - 512 * h:b - 512 * h],
                        v_sb[:, j, 65 * gg:65 * gg + 65],
                        p2s[j][:, gg, c0:c1],
                        start=first, stop=last, skip_group_check=True)

            for kind, j in _B_ORDER[h]:
                if kind == "qk":
                    qk_block(j)
                else:
                    pv_block(j, first=(j == jlist[0]), last=(j == jlist[-1]))

            # ---- normalize: y /= sums (row 64); recips at partitions 0/32 ----
            rs = work.tile([33, 512], f32r, name="rs", tag="rs")
            with nc.allow_low_precision("f32r 1/sums"):
                nc.vector.reciprocal(rs[0:1, :], y_ps[0][64:65, :])
                nc.vector.reciprocal(rs[32:33, :], y_ps[1][64:65, :])
            rb_ps = pmm.tile([128, 512], f32, name="rb_ps", tag="mm")
            nc.tensor.matmul(rb_ps[:], indb_sb[:], rs[:],
                             start=True, stop=True)
            rbc = work.tile([128, 512], f32, name="rbc", tag="rbc")
            nc.scalar.activation(rbc[:], rb_ps[:], Copy)
            for gg in range(2):
                nc.vector.tensor_mul(yTf[r][ts(gg, 64), tsl],
                                     y_ps[gg][0:64, :], rbc[ts(gg, 64), :])

        # ================= phase C(ct, h): output projection ================
        def phase_C(ct, h):
            tsl = slice(512 * h, 512 * h + 512)
            o_ps = pmm.tile([128, 512], f32, name="o_ps", tag="mm")
            for kr in range(4):
                nc.tensor.matmul(o_ps[:], wo_sb[:, kr, ts(ct, 128)],
                                 yTf[kr][:, tsl], start=(kr == 0),
                                 stop=(kr == 3))
            if (ct + h) % 2 == 0:
                nc.scalar.activation(ostage[:, ct, tsl], o_ps[:], Copy)
            else:
                nc.vector.tensor_copy(ostage[:, ct, tsl], o_ps[:])

        # ============================ schedule ==============================
        # A(h0) fully; then A(h1) pieces interleaved into B(h0)'s Act-heavy
        # window; C(.,h0) interleaved into B(h1); C(.,h1) trails.
        A_k(0)
        A_v(0)
        A_q(0, (0, 1))
        A_q(0, (2, 3))
        A_tail(0)
        phase_B(0, 0)
        A_k(1)
        phase_B(0, 1)
        A_v(1)
        phase_B(0, 2)
        A_q(1, (0, 1))
        phase_B(0, 3)
        A_q(1, (2, 3))
        A_tail(1)
        phase_C(0, 0)
        phase_C(1, 0)
        phase_B(1, 0)
        phase_C(2, 0)
        phase_B(1, 1)
        phase_C(3, 0)
        phase_C(4, 0)
        phase_B(1, 2)
        phase_C(5, 0)
        phase_C(6, 0)
        phase_B(1, 3)
        phase_C(7, 0)
        nc.sync.dma_start(outb[:, 0:4, 0:512], ostage[:, 0:4, 0:512])
        nc.sync.dma_start(outb[:, 4:8, 0:512], ostage[:, 4:8, 0:512])
        for ct in range(8):
            phase_C(ct, 1)
            if ct % 2 == 1:
                nc.sync.dma_start(outb[:, ct - 1:ct + 1, 512:1024],
                                  ostage[:, ct - 1:ct + 1, 512:1024])

        if debug:
            for r in range(4):
                nc.sync.dma_start(dbg["d_qTf"][ts(r, 128), :], qTf[r][:])
                nc.sync.dma_start(dbg["d_yTf"][ts(r, 128), :], yTf[r][:])
            nc.sync.dma_start(dbg["d_kTf"][:], kTf[:])
            nc.sync.dma_start(dbg["d_vsb"][:], v_sb[:])
            for h in range(2):
                nc.sync.dma_start(dbg["d_rsq"][:, ts(h, 512)], rsq[h][:])
                nc.sync.dma_start(dbg["d_rkt"][:, 2 * h:2 * h + 2, :],
                                  rk_t[h][:])

    nc.compile()
    _fix_act_table_loads(nc, mybir)
    return nc


def _fix_act_table_loads(nc, mybir):
    """The act-table chooser greedily alternates between exp_and_others and
    natural_log per function, reloading tables 9x. Every activation we emit
    (Exp/Ln/Copy/is_finite) lives in natural_log_exp_and_others, so point all
    loads there and drop the now-redundant reloads."""
    from concourse.hw_specs import get_activation_tables
    tables = list(get_activation_tables(nc.m.arch).items())
    target = next(i for i, (nm, _) in enumerate(tables)
                  if nm == "natural_log_exp_and_others")
    tgt_set = tables[target][1]
    for blk in nc.m.functions[0].blocks:
        for inst in blk.instructions:
            if isinstance(inst, mybir.InstActivation):
                assert inst.func in tgt_set, f"{inst.func} not in target set"
    for blk in nc.m.functions[0].blocks:
        seen = False
        drop = []
        for idx, inst in enumerate(blk.instructions):
            if isinstance(inst, mybir.InstLoadActFuncSet):
                si = inst.sync_info
                has_sync = si is not None and (
                    len(si.on_wait) > 0 or len(si.on_update) > 0)
                if seen and not has_sync:
                    drop.append(idx)
                    continue
                inst.act_func_set_id = target
                seen = True
        for idx in reversed(drop):
            del blk.instructions[idx]


def _rope_interleave(a):
    """Reorder rows of a (64, ...) block so rope pairs (i, i+32) are adjacent
    rows (2i, 2i+1)."""
    out = np.empty_like(a)
    out[0::2] = a[0:32]
    out[1::2] = a[32:64]
    return out


def _const_inputs():
    import ml_dtypes
    bf16 = ml_dtypes.bfloat16
    indq8 = np.zeros((128, 4, 8), dtype=np.float32)
    for r in range(4):
        indq8[0:64, r, 2 * r] = 1.0 / D
        indq8[64:128, r, 2 * r + 1] = 1.0 / D
    ind2g = np.zeros((128, 2), dtype=np.float32)
    ind2g[0:64, 0] = 1.0 / D
    ind2g[64:128, 1] = 1.0 / D
    ind018 = np.zeros((8, 4, 128), dtype=np.float32)
    for r in range(4):
        ind018[2 * r, r, 0:64] = QK_SCALE
        ind018[2 * r + 1, r, 64:128] = QK_SCALE
    indb = np.zeros((33, 128), dtype=np.float32)
    indb[0, 0:64] = 1.0
    indb[32, 64:128] = 1.0
    epsb = np.full((128, 1), EPS, dtype=np.float32)
    return dict(indq8=indq8.astype(bf16),
                ind2g=ind2g.astype(bf16), ind018=ind018, indb=indb,
                epsb=epsb)


def _prep_tables(cos, sin):
    """cosb/sinp [128, T] bf16 tables with rope-interleaved row order."""
    import ml_dtypes
    bf16 = ml_dtypes.bfloat16
    c = np.asarray(cos, dtype=np.float32).reshape(T, D // 2).T   # (32, T)
    s = np.asarray(sin, dtype=np.float32).reshape(T, D // 2).T
    cos64 = np.empty((64, T), dtype=np.float32)
    cos64[0::2] = c
    cos64[1::2] = c
    sin64 = np.empty((64, T), dtype=np.float32)
    sin64[0::2] = -s     # row 2i gets partner u[2i+1]; y1 needs +sin later
    sin64[1::2] = s
    cosb = np.tile(cos64, (2, 1))
    sinp = np.tile(sin64, (2, 1))
    return cosb.astype(bf16), sinp.astype(bf16)


def _prep_core_inputs(x, ve3, cosb, sinp, Wq, Wk, Wv, Wo, Wg, consts, b, s):
    import ml_dtypes
    bf16 = ml_dtypes.bfloat16
    g0, g1 = 2 * s, 2 * s + 1
    xt = np.ascontiguousarray(
        x[b].T.reshape(8, 128, T).transpose(1, 0, 2))          # (128, 8, T)

    Wq4 = Wq.reshape(HKV, REP, D, C)
    wq_rows = np.concatenate(
        [_rope_interleave(Wq4[g, r]) for r in range(REP) for g in (g0, g1)],
        axis=0)                                                # (512, C)
    wq = np.ascontiguousarray(
        wq_rows.T.reshape(8, 128, 512).transpose(1, 0, 2))     # (128, 8, 512)
    Wk3 = Wk.reshape(HKV, D, C)
    wk_rows = np.concatenate(
        [_rope_interleave(Wk3[g]) for g in (g0, g1)], axis=0)  # (128, C)
    wk = np.ascontiguousarray(
        wk_rows.T.reshape(8, 128, 128).transpose(1, 0, 2))
    Wv3 = Wv.reshape(HKV, D, C)
    wv_rows = np.concatenate([Wv3[g0], Wv3[g1]], axis=0)
    wv = np.ascontiguousarray(
        wv_rows.T.reshape(8, 128, 128).transpose(1, 0, 2))

    Wo4 = Wo.reshape(C, HKV, REP, D)
    wo_cols = np.concatenate([Wo4[:, g, r, :] for r in range(REP)
                              for g in (g0, g1)], axis=1)      # (C, 512)
    wo = np.ascontiguousarray(
        wo_cols.T.reshape(4, 128, C).transpose(1, 0, 2))       # (128, 4, C)

    wg = np.zeros((16, 2), dtype=np.float32)
    wg[0:GATE_CH, 0] = Wg[g0]
    wg[0:GATE_CH, 1] = Wg[g1]

    ve4 = ve3[b].reshape(T, HKV, D)
    vet = np.concatenate([ve4[:, g0, :], ve4[:, g1, :]], axis=1)  # (T, 128)
    vet = np.ascontiguousarray(
        vet.reshape(8, 128, 128).transpose(1, 0, 2))              # (128, 8, 128)

    d = dict(xt=xt.astype(bf16), wq=wq.astype(bf16), wk=wk.astype(bf16),
             wv=wv.astype(bf16), wo=wo.astype(bf16), wg=wg.astype(bf16),
             vet=vet.astype(bf16), cosb=cosb, sinp=sinp)
    d.update(consts)
    return d


def kernel(x, ve, cos, sin, Wq, Wk, Wv, Wo, Wg, window_size):
    from concourse.bass_utils import run_bass_kernel_spmd

    assert int(window_size) == WINDOW
    x = np.asarray(x, dtype=np.float32)
    ve3 = 3.0 * np.asarray(ve, dtype=np.float32)
    Wq = np.asarray(Wq, dtype=np.float32)
    Wk = np.asarray(Wk, dtype=np.float32)
    Wv = np.asarray(Wv, dtype=np.float32)
    Wo = np.asarray(Wo, dtype=np.float32)
    Wg = np.asarray(Wg, dtype=np.float32)
    cosb, sinp = _prep_tables(cos, sin)
    consts = _const_inputs()

    if "nc" not in _CACHE:
        _CACHE["nc"] = _build_program()
    nc = _CACHE["nc"]

    in_maps = []
    for core in range(NCORES):
        b, s = core // 2, core % 2
        in_maps.append(_prep_core_inputs(x, ve3, cosb, sinp,
                                         Wq, Wk, Wv, Wo, Wg, consts, b, s))

    res = run_bass_kernel_spmd(nc, in_maps, core_ids=list(range(NCORES)))
    out = np.empty((B, T, C), dtype=np.float32)
    for b in range(B):
        acc = (np.asarray(res.results[2 * b]["outb"], dtype=np.float32)
               + np.asarray(res.results[2 * b + 1]["outb"], dtype=np.float32))
        acc = acc.reshape(128, 8, T).transpose(1, 0, 2).reshape(C, T)
        out[b] = acc.T
    return out
